# revision 1
# baseline (speedup 1.0000x reference)
"""Trainium2 Bass kernel for nn_Block_24111946399747 (dense transformer block).

Strategy (8 NeuronCores, two SPMD launches — on-device collectives are
unavailable on this terminal, so the heads->rows reshard happens on host):

Launch 1 (head-sharded attention; core i owns heads 2i, 2i+1):
  - Every core computes LayerNorm(x) over all 4096 tokens (LN affine params
    are folded into the QKV weights on the host).
  - QKV in feature-major layout so no on-chip transposes are needed:
      q_t, k_t : [HD, T] per head   (out^T = w^T-stationary arrangement)
      v        : [T, HD] token-major
      S^T      : [tk, tq] = k_t.T @ q_t ;  P^T = exp(S^T/sqrt(HD)) (causal)
      y_t      : [HD, tq] = v^T @ P^T   (rowsum via ones-matmul, then scale)
  - Returns y_t [256, 4096] bf16 (this core's two heads, all tokens).

Host: stack per-core y_t -> y_all [2048, 4096] (features in head order =
w_proj row order), hand each core its 512-token column slice.

Launch 2 (row-sharded; core i owns token rows 512i..512i+512):
  - x2 = y^T @ w_proj + b_proj + x   (token-major out)
  - out = gelu_tanh(x2 @ w_fc + b_fc) @ w_out + b_out + x2
  - Returns out [512, 2048] f32; host concatenates.

Matmuls in bf16 with fp32 PSUM accumulation; LN stats, softmax sums and both
residual adds in fp32.
"""

import math
import os
import sys

import numpy as np

if "/opt/trn_rl_repo" not in sys.path:
    sys.path.insert(0, "/opt/trn_rl_repo")

import ml_dtypes  # noqa: E402

import concourse.bass as bass  # noqa: E402,F401
import concourse.mybir as mybir  # noqa: E402
import concourse.tile as tile  # noqa: E402
from concourse import bacc  # noqa: E402
from concourse.bass_utils import run_bass_kernel_spmd  # noqa: E402

B, T, C, H = 2, 2048, 2048, 16
HD = C // H            # 128 head dim
N_CORES = 8
HPC = H // N_CORES     # 2 heads per core
NTOK = B * T           # 4096 tokens
RPC = NTOK // N_CORES  # 512 rows per core
P = 128
KC = C // P            # 16 contraction chunks over C
F4 = 4 * C             # 8192
MC4 = F4 // P          # 64 contraction chunks over 4C
NT = NTOK // P         # 32 token tiles of 128
NTT = NTOK // 512      # 8 token tiles of 512
JTT = T // 512         # 4 query tiles of 512 per batch
EPS = 1e-6
BF16 = mybir.dt.bfloat16
F32 = mybir.dt.float32
AX = mybir.AxisListType
ALU = mybir.AluOpType
ACT = mybir.ActivationFunctionType

_BUILT1 = None
_BUILT2 = None
# Phase gating for bisection: prefix of "ABC" (launch 1) / "DE" (launch 2).
_PHASES = os.environ.get("KERNEL_PHASES", "ABCDE")
# "lut": single ACT Gelu op (HW only); "exact": sigmoid-identity formula
# matching the reference's tanh approximation (CoreSim-compatible).
_GELU = os.environ.get("KERNEL_GELU", "lut")


# ======================= Launch 1: LN + QKV + attention =======================

def _emit_attn(nc, tc, io):
    x_full = io["x_full"]
    w_qk, b_qk, w_v, b_v = io["w_qk"], io["b_qk"], io["w_v"], io["b_v"]
    cmask, yt_out = io["cmask"], io["yt"]
    ph = _PHASES

    from contextlib import ExitStack

    with ExitStack() as es:
        constp = es.enter_context(tc.tile_pool(name="constp", bufs=1))
        dramp = es.enter_context(tc.tile_pool(name="dramp", bufs=1,
                                              space="DRAM"))
        ones_sb = constp.tile([P, 1], BF16, name="ones_sb")
        nc.any.memset(ones_sb[:], 1.0)
        eps_sb = constp.tile([P, 1], F32, name="eps_sb")
        nc.any.memset(eps_sb[:], EPS)
        mask_sb = constp.tile([P, P], BF16, name="mask_sb")
        nc.sync.dma_start(mask_sb[:], cmask[:, :])
        b_qk_sb = constp.tile([P, 4], F32, name="b_qk_sb")
        nc.sync.dma_start(b_qk_sb[:], b_qk.ap().rearrange("(c p) -> p c", p=P))
        b_v_sb = constp.tile([P, HPC * HD], F32, name="b_v_sb")
        nc.sync.dma_start(b_v_sb[:], b_v.ap()[None, :].to_broadcast((P, HPC * HD)))

        # per-tt DRAM tiles so QKV transposes start as soon as each
        # 512-token block of LN output lands (whole-tile dep granularity)
        h_bfs = [dramp.tile([512, C], BF16, name=f"h_bf{tt}")
                 for tt in range(NTT)]

        # QKV-phase SBUF allocated BEFORE the LN pools: address aliasing with
        # LN tiles would add WAR deps serializing QKV behind the whole of LN.
        persbc = es.enter_context(tc.tile_pool(name="persbc", bufs=1))
        qk_t = persbc.tile([P, 4, NTOK], BF16, name="qk_t")
        v_sb = persbc.tile([P, NT, HPC * HD], BF16, name="v_sb")
        wqp = es.enter_context(tc.tile_pool(name="wqp", bufs=1))
        w_qk_sb = wqp.tile([P, KC, 4 * P], BF16, name="w_qk_sb")
        nc.sync.dma_start(
            w_qk_sb[:], w_qk.ap().rearrange("(ko p) f -> p ko f", p=P))
        w_v_sb = wqp.tile([P, KC, HPC * HD], BF16, name="w_v_sb")
        nc.sync.dma_start(
            w_v_sb[:], w_v.ap().rearrange("(ko p) f -> p ko f", p=P))
        hbp = es.enter_context(tc.tile_pool(name="hbp", bufs=2))

        # ---------------- Phase A: LayerNorm ----------------
        # mean+var in one DVE pass via bn_stats (4x512 subgroups + bn_aggr);
        # sqrt/reciprocal batched per group of 4 tiles (one 512-token block).
        GRP = 4
        with tc.tile_pool(name="lnp", bufs=GRP + 2) as lnp, \
             tc.tile_pool(name="lnw", bufs=3) as lnw, \
             tc.tile_pool(name="lns", bufs=2) as lns:
            for g in range(NT // GRP):
                xts = []
                mvg = lns.tile([P, GRP, 2], F32, tag="mvg")
                for j in range(GRP):
                    t = g * GRP + j
                    xt = lnp.tile([P, C], F32, tag="xt")
                    nc.sync.dma_start(xt[:], x_full[t * P:(t + 1) * P, :])
                    xts.append(xt)
                    stats = lnw.tile([P, 4, 6], F32, tag="stats")
                    xr = xt[:].rearrange("p (s f) -> p s f", f=512)
                    for s in range(4):
                        nc.vector.bn_stats(stats[:, s, :], xr[:, s, :])
                    nc.vector.bn_aggr(mvg[:, j, :], stats[:])
                stdg = lns.tile([P, GRP], F32, tag="stdg")
                nc.scalar.activation(stdg[:], mvg[:, :, 1], ACT.Sqrt,
                                     bias=eps_sb[:])
                rstdg = lns.tile([P, GRP], F32, tag="rstdg")
                nc.vector.reciprocal(rstdg[:], stdg[:])
                nmrg = lns.tile([P, GRP], F32, tag="nmrg")
                nc.vector.tensor_mul(nmrg[:], mvg[:, :, 0], rstdg[:])
                nc.vector.tensor_scalar_mul(nmrg[:], nmrg[:], -1.0)
                for j in range(GRP):
                    t = g * GRP + j
                    ht = lnw.tile([P, C], BF16, tag="ht")
                    nc.scalar.activation(ht[:], xts[j][:], ACT.Identity,
                                         bias=nmrg[:, j:j + 1],
                                         scale=rstdg[:, j:j + 1])
                    nc.sync.dma_start(
                        h_bfs[t // GRP][(t % GRP) * P:(t % GRP + 1) * P, :],
                        ht[:])

        if "B" not in ph:  # dump LN output rows into yt and stop
            with tc.tile_pool(name="dmp", bufs=2) as dmp:
                for rb in range(2):
                    t = dmp.tile([P, C], BF16, tag="t")
                    nc.sync.dma_start(t[:], h_bfs[0][rb * P:(rb + 1) * P, :])
                    nc.sync.dma_start(yt_out[rb * P:(rb + 1) * P, :C], t[:])
            return

        # ---------------- Phase B: QKV projections ----------------
        # qps kept small (3 PSUM banks) so attention's PSUM pools fit
        # alongside without address aliasing (which would serialize phases).
        with tc.tile_pool(name="qps", bufs=4, space="PSUM") as qps:
            for tt in range(NTT):
                h_t = hbp.tile([P, KC, 512], BF16, tag="h_t")
                for ko in range(KC):
                    nc.sync.dma_start_transpose(
                        h_t[:, ko, :],
                        h_bfs[tt][:, ko * P:(ko + 1) * P])
                for fc in range(4):  # q0,q1,k0,k1 feature chunks
                    ps = qps.tile([P, 512], F32, tag="qk_ps")
                    for ko in range(KC):
                        nc.tensor.matmul(
                            ps[:], w_qk_sb[:, ko, fc * P:(fc + 1) * P],
                            h_t[:, ko, :],
                            start=(ko == 0), stop=(ko == KC - 1))
                    nc.scalar.activation(
                        qk_t[:, fc, tt * 512:(tt + 1) * 512], ps[:],
                        ACT.Identity, bias=b_qk_sb[:, fc:fc + 1], scale=1.0)
                for t2 in range(4):  # 128-token chunks, token-major v
                    psv = qps.tile([P, HPC * HD], F32, tag="v_ps")
                    for ko in range(KC):
                        nc.tensor.matmul(
                            psv[:], h_t[:, ko, t2 * P:(t2 + 1) * P],
                            w_v_sb[:, ko, :],
                            start=(ko == 0), stop=(ko == KC - 1))
                    nc.vector.tensor_add(
                        v_sb[:, tt * 4 + t2, :], psv[:], b_v_sb[:, :])

        if "C" not in ph:  # dump q_t head 0 into yt and stop
            with tc.tile_pool(name="dmp", bufs=2) as dmp:
                for rb in range(2):
                    t = dmp.tile([P, NTOK], BF16, tag="t")
                    nc.vector.tensor_copy(t[:], qk_t[:, rb, :])
                    nc.sync.dma_start(yt_out[rb * P:(rb + 1) * P, :], t[:])
            return

        # ---------------- Phase C: causal attention ----------------
        with tc.tile_pool(name="sps", bufs=3, space="PSUM") as sps, \
             tc.tile_pool(name="yps", bufs=2, space="PSUM") as yps, \
             tc.tile_pool(name="rps", bufs=2, space="PSUM") as rps, \
             tc.tile_pool(name="attp", bufs=2) as attp:
            inv_sqrt_hd = 1.0 / math.sqrt(HD)
            for b in range(B):
                for hl in range(HPC):
                    q_sl = qk_t[:, hl, b * T:(b + 1) * T]
                    k_sl = qk_t[:, 2 + hl, b * T:(b + 1) * T]
                    for jt in range(JTT):
                        nblk = 4 * (jt + 1)
                        pt = attp.tile([P, 16, 512], BF16, tag="pt")
                        y_ps = yps.tile([P, 512], F32, tag="y_ps")
                        rs_ps = rps.tile([1, 512], F32, tag="rs_ps")
                        for ib in range(nblk):
                            c0 = max(0, ib * P - jt * 512)
                            s_ps = sps.tile([P, 512], F32, tag="s_ps")
                            nc.tensor.matmul(
                                s_ps[:, c0:512],
                                k_sl[:, ib * P:(ib + 1) * P],
                                q_sl[:, jt * 512 + c0:(jt + 1) * 512],
                                start=True, stop=True)
                            nc.scalar.activation(
                                pt[:, ib, c0:512], s_ps[:, c0:512],
                                ACT.Exp, scale=inv_sqrt_hd)
                            if ib >= 4 * jt:  # diagonal 128x128 sub-block
                                nc.vector.tensor_mul(
                                    pt[:, ib, c0:c0 + P],
                                    pt[:, ib, c0:c0 + P], mask_sb[:])
                            vv = v_sb[:, b * (T // P) + ib,
                                      hl * HD:(hl + 1) * HD]
                            nc.tensor.matmul(
                                y_ps[:, c0:512], vv, pt[:, ib, c0:512],
                                start=(ib == 0), stop=(ib == nblk - 1))
                            nc.tensor.matmul(
                                rs_ps[:, c0:512], ones_sb[:],
                                pt[:, ib, c0:512],
                                start=(ib == 0), stop=(ib == nblk - 1))
                        rsv = attp.tile([1, 512], F32, tag="rsv")
                        nc.vector.reciprocal(rsv[:], rs_ps[:])
                        rbc = attp.tile([P, 512], F32, tag="rbc")
                        nc.gpsimd.partition_broadcast(rbc[:], rsv[:])
                        y_bf = attp.tile([P, 512], BF16, tag="y_bf")
                        nc.vector.tensor_mul(y_bf[:], y_ps[:], rbc[:])
                        nc.sync.dma_start(
                            yt_out[hl * HD:(hl + 1) * HD,
                                   b * T + jt * 512:b * T + (jt + 1) * 512],
                            y_bf[:])


def _build_attn():
    nc = bacc.Bacc("TRN2", target_bir_lowering=False, debug=False,
                   num_devices=N_CORES)
    io = {}
    io["x_full"] = nc.dram_tensor("x_full", [NTOK, C], F32,
                                  kind="ExternalInput").ap()
    io["w_qk"] = nc.dram_tensor("w_qk", [C, 2 * HPC * HD], BF16,
                                kind="ExternalInput")
    io["b_qk"] = nc.dram_tensor("b_qk", [2 * HPC * HD], F32,
                                kind="ExternalInput")
    io["w_v"] = nc.dram_tensor("w_v", [C, HPC * HD], BF16,
                               kind="ExternalInput")
    io["b_v"] = nc.dram_tensor("b_v", [HPC * HD], F32, kind="ExternalInput")
    io["cmask"] = nc.dram_tensor("cmask", [P, P], BF16,
                                 kind="ExternalInput").ap()
    io["yt"] = nc.dram_tensor("yt", [HPC * HD, NTOK], BF16,
                              kind="ExternalOutput").ap()
    with tile.TileContext(nc) as tc:
        _emit_attn(nc, tc, io)
    nc.compile()
    return nc


# ======================= Launch 2: proj + MLP =======================

def _emit_mlp(nc, tc, io):
    y_t_in, x_rows = io["y_t"], io["x_rows"]
    w_pr, b_pr = io["w_pr"], io["b_pr"]
    w_fc, b_fc, w_out, b_out = io["w_fc"], io["b_fc"], io["w_out"], io["b_out"]
    out = io["out"]
    ph = _PHASES

    from contextlib import ExitStack

    with ExitStack() as es:
        constp = es.enter_context(tc.tile_pool(name="constp", bufs=1))
        dramp = es.enter_context(tc.tile_pool(name="dramp", bufs=1,
                                              space="DRAM"))
        b_fc_sb = constp.tile([P, MC4], F32, name="b_fc_sb")
        nc.sync.dma_start(b_fc_sb[:], b_fc.ap().rearrange("(c p) -> p c", p=P))
        b_pr_sb = constp.tile([P, C], F32, name="b_pr_sb")
        nc.sync.dma_start(b_pr_sb[:], b_pr.ap()[None, :].to_broadcast((P, C)))
        b_out_sb = constp.tile([P, C], F32, name="b_out_sb")
        nc.sync.dma_start(b_out_sb[:], b_out.ap()[None, :].to_broadcast((P, C)))

        # per-ct DRAM tiles so x2 transposes start per 512-col block
        x2_bfs = [dramp.tile([RPC, 512], BF16, name=f"x2_bf{ct}")
                  for ct in range(4)]

        persde = es.enter_context(tc.tile_pool(name="persde", bufs=1))
        x2_f32 = persde.tile([P, 4, C], F32, name="x2_f32")
        x2_t = persde.tile([P, KC, RPC], BF16, name="x2_t")

        # ---------------- Phase D: proj + residual ----------------
        with tc.tile_pool(name="pdp", bufs=2) as pdp, \
             tc.tile_pool(name="wprp", bufs=2) as wprp, \
             tc.tile_pool(name="dps", bufs=4, space="PSUM") as dps:
            y_all = pdp.tile([P, KC, RPC], BF16, name="y_all")
            nc.sync.dma_start(
                y_all[:], y_t_in[:, :].rearrange("(ko p) r -> p ko r", p=P))
            for rb in range(4):
                nc.sync.dma_start(
                    x2_f32[:, rb, :], x_rows[rb * P:(rb + 1) * P, :])
            for ct in range(4):
                wt = wprp.tile([P, KC, 512], BF16, tag="wpr")
                nc.sync.dma_start(
                    wt[:],
                    w_pr.ap()[:, ct * 512:(ct + 1) * 512]
                    .rearrange("(ko p) f -> p ko f", p=P))
                for rb in range(4):
                    ps = dps.tile([P, 512], F32, tag="pr_ps")
                    for ko in range(KC):
                        nc.tensor.matmul(
                            ps[:], y_all[:, ko, rb * P:(rb + 1) * P],
                            wt[:, ko, :],
                            start=(ko == 0), stop=(ko == KC - 1))
                    sl = x2_f32[:, rb, ct * 512:(ct + 1) * 512]
                    nc.vector.tensor_add(sl, sl, ps[:])
                    nc.vector.tensor_add(
                        sl, sl, b_pr_sb[:, ct * 512:(ct + 1) * 512])
                    x2b = pdp.tile([P, 512], BF16, tag="x2b")
                    nc.vector.tensor_copy(x2b[:], sl)
                    nc.sync.dma_start(
                        x2_bfs[ct][rb * P:(rb + 1) * P, :], x2b[:])
                # transpose-load this ct's four feature chunks immediately
                for kk in range(4):
                    nc.sync.dma_start_transpose(
                        x2_t[:, 4 * ct + kk, :],
                        x2_bfs[ct][:, kk * P:(kk + 1) * P])

        if "E" not in ph:  # dump x2 and stop
            with tc.tile_pool(name="dmp", bufs=2) as dmp:
                for rb in range(4):
                    nc.sync.dma_start(
                        out[rb * P:(rb + 1) * P, :], x2_f32[:, rb, :])
            return

        # ---------------- Phase E: MLP + residual ----------------
        with tc.tile_pool(name="mep", bufs=1) as mep, \
             tc.tile_pool(name="wfcp", bufs=4) as wfcp, \
             tc.tile_pool(name="wop", bufs=8) as wop, \
             tc.tile_pool(name="glp", bufs=3) as glp, \
             tc.tile_pool(name="ofp", bufs=3) as ofp, \
             tc.tile_pool(name="eps1", bufs=3, space="PSUM") as eps1, \
             tc.tile_pool(name="eps2", bufs=1, space="PSUM") as eps2:
            m_sb = mep.tile([P, MC4, RPC], BF16, name="m_sb")
            # gelu_tanh(u) = 0.5u(1+tanh(c(u+0.044715u^3))) = u*sigmoid(2c*u*(1+0.044715u^2))
            two_c = 2.0 * math.sqrt(2.0 / math.pi)
            for mc in range(MC4):
                wfc_t = wfcp.tile([P, KC, P], BF16, tag="wfc")
                nc.sync.dma_start(
                    wfc_t[:],
                    w_fc.ap()[:, mc * P:(mc + 1) * P]
                    .rearrange("(ko p) m -> p ko m", p=P))
                ps = eps1.tile([P, 512], F32, tag="fc_ps")
                for ko in range(KC):
                    nc.tensor.matmul(
                        ps[:], wfc_t[:, ko, :], x2_t[:, ko, :],
                        start=(ko == 0), stop=(ko == KC - 1))
                if _GELU == "lut":
                    nc.scalar.activation(
                        m_sb[:, mc, :], ps[:], ACT.Gelu,
                        bias=b_fc_sb[:, mc:mc + 1], scale=1.0)
                else:
                    u = glp.tile([P, 512], F32, tag="gl_u")
                    nc.scalar.activation(u[:], ps[:], ACT.Identity,
                                         bias=b_fc_sb[:, mc:mc + 1],
                                         scale=1.0)
                    t = glp.tile([P, 512], F32, tag="gl_t")
                    nc.vector.tensor_mul(t[:], u[:], u[:])
                    nc.vector.tensor_scalar(t[:], t[:], 0.044715, 1.0,
                                            op0=ALU.mult, op1=ALU.add)
                    nc.vector.tensor_mul(t[:], t[:], u[:])
                    sg = glp.tile([P, 512], F32, tag="gl_sg")
                    nc.scalar.activation(sg[:], t[:], ACT.Sigmoid,
                                         scale=two_c)
                    nc.vector.tensor_mul(m_sb[:, mc, :], u[:], sg[:])
            for ct in range(4):
                pss = [eps2.tile([P, 512], F32, tag=f"o_ps{rb}",
                                 name=f"o_ps{rb}_{ct}")
                       for rb in range(4)]
                for ko in range(MC4):
                    wo_t = wop.tile([P, 512], BF16, tag="wo")
                    nc.sync.dma_start(
                        wo_t[:],
                        w_out.ap()[ko * P:(ko + 1) * P,
                                   ct * 512:(ct + 1) * 512])
                    for rb in range(4):
                        nc.tensor.matmul(
                            pss[rb][:], m_sb[:, ko, rb * P:(rb + 1) * P],
                            wo_t[:],
                            start=(ko == 0), stop=(ko == MC4 - 1))
                for rb in range(4):
                    of = ofp.tile([P, 512], F32, tag="of")
                    nc.vector.tensor_add(
                        of[:], pss[rb][:],
                        x2_f32[:, rb, ct * 512:(ct + 1) * 512])
                    nc.vector.tensor_add(
                        of[:], of[:], b_out_sb[:, ct * 512:(ct + 1) * 512])
                    nc.sync.dma_start(
                        out[rb * P:(rb + 1) * P, ct * 512:(ct + 1) * 512],
                        of[:])


def _build_mlp():
    nc = bacc.Bacc("TRN2", target_bir_lowering=False, debug=False,
                   num_devices=N_CORES)
    io = {}
    io["y_t"] = nc.dram_tensor("y_t", [C, RPC], BF16, kind="ExternalInput")
    io["x_rows"] = nc.dram_tensor("x_rows", [RPC, C], F32,
                                  kind="ExternalInput").ap()
    io["w_pr"] = nc.dram_tensor("w_pr", [C, C], BF16, kind="ExternalInput")
    io["b_pr"] = nc.dram_tensor("b_pr", [C], F32, kind="ExternalInput")
    io["w_fc"] = nc.dram_tensor("w_fc", [C, F4], BF16, kind="ExternalInput")
    io["b_fc"] = nc.dram_tensor("b_fc", [F4], F32, kind="ExternalInput")
    io["w_out"] = nc.dram_tensor("w_out", [F4, C], BF16, kind="ExternalInput")
    io["b_out"] = nc.dram_tensor("b_out", [C], F32, kind="ExternalInput")
    io["out"] = nc.dram_tensor("out", [RPC, C], F32,
                               kind="ExternalOutput").ap()
    with tile.TileContext(nc) as tc:
        _emit_mlp(nc, tc, io)
    nc.compile()
    return nc


def _get_built():
    global _BUILT1, _BUILT2
    if _BUILT1 is None:
        _BUILT1 = _build_attn()
    if _BUILT2 is None and any(p in _PHASES for p in "DE"):
        _BUILT2 = _build_mlp()
    return _BUILT1, _BUILT2


# ======================= Host orchestration =======================

def _prep(x, ln_scale, ln_bias, w_qkv, b_qkv, w_proj, b_proj,
          w_fc, b_fc, w_out, b_out):
    bf = ml_dtypes.bfloat16
    xf = np.ascontiguousarray(np.asarray(x, np.float32).reshape(NTOK, C))
    # Fold LN affine into the QKV projection (exact, in float64).
    w64 = np.asarray(w_qkv, np.float64)
    g = np.asarray(ln_scale, np.float64)
    beta = np.asarray(ln_bias, np.float64)
    w_eff = g[:, None] * w64
    b_eff = np.asarray(b_qkv, np.float64) + beta @ w64

    wq, wk, wv = w_eff[:, :C], w_eff[:, C:2 * C], w_eff[:, 2 * C:]
    bq, bk, bv = b_eff[:C], b_eff[C:2 * C], b_eff[2 * C:]
    cmask = np.triu(np.ones((P, P), np.float32)).astype(bf)

    in1 = []
    for i in range(N_CORES):
        hs = slice(i * HPC * HD, (i + 1) * HPC * HD)
        w_qk_i = np.ascontiguousarray(
            np.concatenate([wq[:, hs], wk[:, hs]], axis=1).astype(np.float32)
        ).astype(bf)
        b_qk_i = np.ascontiguousarray(
            np.concatenate([bq[hs], bk[hs]]).astype(np.float32))
        w_v_i = np.ascontiguousarray(wv[:, hs].astype(np.float32)).astype(bf)
        b_v_i = np.ascontiguousarray(bv[hs].astype(np.float32))
        in1.append({
            "x_full": xf,
            "w_qk": w_qk_i, "b_qk": b_qk_i, "w_v": w_v_i, "b_v": b_v_i,
            "cmask": cmask,
        })

    w_pr_b = np.asarray(w_proj, np.float32).astype(bf)
    w_fc_b = np.asarray(w_fc, np.float32).astype(bf)
    w_out_b = np.asarray(w_out, np.float32).astype(bf)
    b_pr_f = np.ascontiguousarray(np.asarray(b_proj, np.float32))
    b_fc_f = np.ascontiguousarray(np.asarray(b_fc, np.float32))
    b_out_f = np.ascontiguousarray(np.asarray(b_out, np.float32))
    in2_common = {
        "w_pr": w_pr_b, "b_pr": b_pr_f, "w_fc": w_fc_b, "b_fc": b_fc_f,
        "w_out": w_out_b, "b_out": b_out_f,
    }
    return xf, in1, in2_common


def run(inputs, trace=False, trace_cores=None):
    """Run both SPMD launches. Returns (output [B,T,C] f32, res1, res2)."""
    nc1, nc2 = _get_built()
    xf, in1, in2_common = _prep(**inputs)
    kwargs = {}
    if trace:
        kwargs = dict(trace=True,
                      trace_cores=trace_cores if trace_cores else [0])
    res1 = run_bass_kernel_spmd(nc1, in1, core_ids=list(range(N_CORES)),
                                **kwargs)
    y_all = np.concatenate(
        [np.asarray(res1.results[i]["yt"]) for i in range(N_CORES)], axis=0)
    if nc2 is None:
        return y_all, res1, None

    in2 = []
    for i in range(N_CORES):
        in2.append({
            "y_t": np.ascontiguousarray(y_all[:, i * RPC:(i + 1) * RPC]),
            "x_rows": np.ascontiguousarray(xf[i * RPC:(i + 1) * RPC]),
            **in2_common,
        })
    res2 = run_bass_kernel_spmd(nc2, in2, core_ids=list(range(N_CORES)),
                                **kwargs)
    outf = np.concatenate(
        [np.asarray(res2.results[i]["out"]) for i in range(N_CORES)], axis=0)
    return outf.reshape(B, T, C).astype(np.float32), res1, res2


def kernel(**inputs):
    out, _, _ = run(inputs, trace=False)
    return out

